# revision 13
# baseline (speedup 1.0000x reference)
"""Phi-2-style attention layer (B=1, L=2048, D=2560, 32 heads, partial rope 32)
as a distributed Bass kernel on 8 TRN2 NeuronCores.

Sharding: tensor-parallel over heads (4 heads/core), x replicated.

Fully software-pipelined single pass per 512-column group g (qc = g):
  QKV projection for cols [512g, 512g+512) -> rope + scatter -> attention for
  qc=g (k blocks 0..4g+3) -> per-group AllGather of the normalized attention
  output attnT [320, 512] -> out-projection of qc=g deferred one group and
  interleaved instruction-by-instruction into attention of qc=g+1 (and across
  rep boundaries), so collective latency is hidden behind tensor work.

PSUM budget (8 banks): QKV projection uses 2 rotating banks (sequential
accumulation groups m0,m1,m2,m3,m4,v0..v3 over all 20 k-tiles each), scores
S^T 2, PV 2 (heads processed outer-loop), out-proj 2.

Masking is multiplicative after exp: est = exp(S^T) (unnormalized softmax,
scores are O(5) so exp never overflows bf16), then est *= exp(mask) where
needed. Causal diagonal blocks multiply only a shared 128x128 triangle tile
and skip fully-masked leading columns (narrowed matmul/exp widths). The
denominator comes from a ones-column appended to V at padded partition 96.
Arbitrary additive masks degrade gracefully to per-block exp(mask) tiles.
"""

import math
from contextlib import ExitStack

import numpy as np

L = 2048
C = 2560
NCORES = 8
HPC = 4          # heads per core
HD = 80          # head dim
DH = HPC * HD    # 320 dims per core
ROT = 32
RD = ROT // 2    # 16
BASE = 10000.0
KT = C // 128    # 20 k tiles
NW = 512         # group width (q cols per group)
NG = L // NW     # 4 groups
SC = 1.0 / math.sqrt(HD)

ZERO, NEG, TRI, MIXG = 0, 1, 2, 3

_RUNNERS = {}


def _rest_runs(mi):
    """For qk m-tile mi in {2,3,4}: contiguous runs (is_q, head, dst_row,
    src_row, nrows) mapping eviction rows to per-head tiles."""
    runs = []
    p = 0
    while p < 128:
        r = (mi - 2) * 128 + p
        if r < 192:
            h, j, is_q = r // 48, r % 48, True
        else:
            h, j, is_q = (r - 192) // 48, (r - 192) % 48, False
        n = min(48 - j, 128 - p)
        runs.append((is_q, h, 32 + j, p, n))
        p += n
    return runs


def _build(mask_plan, reps=1, sim_mode=False):
    import concourse.bacc as bacc
    import concourse.tile as tile
    from concourse import mybir

    f32 = mybir.dt.float32
    f32r = mybir.dt.float32r
    bf16 = mybir.dt.bfloat16
    cls, nmix = mask_plan

    nc = bacc.Bacc("TRN2", target_bir_lowering=False, debug=False,
                   num_devices=NCORES)

    d = {}
    d["xTc"] = nc.dram_tensor("xTc", [NG, C, NW], bf16,
                              kind="ExternalInput").ap()
    d["wqk"] = nc.dram_tensor("wqk_t", [C, 2 * DH], bf16,
                              kind="ExternalInput").ap()
    d["wv"] = nc.dram_tensor("wv_t", [C, DH], bf16, kind="ExternalInput").ap()
    d["wo"] = nc.dram_tensor("wo_t", [C, DH], bf16, kind="ExternalInput").ap()
    d["bqk"] = nc.dram_tensor("bqk", [128, 5], f32, kind="ExternalInput").ap()
    d["cb"] = nc.dram_tensor("cb", [1, DH], f32r, kind="ExternalInput").ap()
    d["cos"] = nc.dram_tensor("cos8", [128, L], bf16,
                              kind="ExternalInput").ap()
    d["sin"] = nc.dram_tensor("sin8", [128, L], bf16,
                              kind="ExternalInput").ap()
    d["ones"] = nc.dram_tensor("ones128", [1, 128], f32r,
                               kind="ExternalInput").ap()
    d["vpad"] = nc.dram_tensor("vpad", [128, 16, HPC, 17], bf16,
                               kind="ExternalInput").ap()
    d["tri"] = nc.dram_tensor("tri128", [128, 128], bf16,
                              kind="ExternalInput").ap()
    d["mixm"] = nc.dram_tensor("mixm", [max(nmix, 1), 128, 512], bf16,
                               kind="ExternalInput").ap()
    d["y"] = nc.dram_tensor("y", [L, DH], bf16, kind="ExternalOutput").ap()

    with tile.TileContext(nc) as tc:
        _emit(nc, tc, mybir, d, cls, nmix, reps, sim_mode)
    nc.compile()
    return nc


def _emit(nc, tc, mybir, d, cls, nmix, reps, sim_mode):
    from concourse import mybir as mb

    f32 = mybir.dt.float32
    f32r = mybir.dt.float32r
    bf16 = mybir.dt.bfloat16
    Exp = mybir.ActivationFunctionType.Exp
    Identity = mybir.ActivationFunctionType.Identity
    Copy = mybir.ActivationFunctionType.Copy
    bypass = mybir.AluOpType.bypass

    wqk_r = d["wqk"].rearrange("(t p) m -> p t m", p=128)
    wv_r = d["wv"].rearrange("(t p) m -> p t m", p=128)
    wo_r = d["wo"].rearrange("(t p) m -> p t m", p=128)
    mix_r = d["mixm"].rearrange("b p q -> p b q")

    live_kb = {qc: [kb for kb in range(16) if cls[kb][qc][0] != NEG]
               for qc in range(NG)}
    # mix block index, qc-major
    mix_idx = {}
    mi = 0
    for qc in range(NG):
        for kb in range(16):
            if cls[kb][qc][0] == MIXG:
                mix_idx[(kb, qc)] = mi
                mi += 1
    qc_mix = {qc: [kb for kb in range(16) if cls[kb][qc][0] == MIXG]
              for qc in range(NG)}

    ctx = ExitStack()
    with ctx:
        small = ctx.enter_context(tc.tile_pool(name="small", bufs=1))
        ones_sb = small.tile([1, 128], f32r, name="ones", tag="ones")
        cb_sb = small.tile([1, DH], f32r, name="cb", tag="cb")
        bqk_sb = small.tile([128, 5], f32, name="bqk", tag="bqk")
        tri_sb = small.tile([128, 128], bf16, name="tri", tag="tri")
        cos_sb = small.tile([128, L], bf16, name="cos", tag="cos")
        sin_sb = small.tile([128, L], bf16, name="sin", tag="sin")

        wpool = ctx.enter_context(tc.tile_pool(name="w", bufs=1))
        wqk_sb = wpool.tile([128, KT, 2 * DH], bf16, name="wqk", tag="wqk")
        wv_sb = wpool.tile([128, KT, DH], bf16, name="wv", tag="wv")
        wo_sb = wpool.tile([128, KT, DH], bf16, name="wo", tag="wo")

        qkpool = ctx.enter_context(tc.tile_pool(name="qk", bufs=1))
        qh = [qkpool.tile([128, L], bf16, name=f"qh{h}", tag=f"qh{h}")
              for h in range(HPC)]
        kh = [qkpool.tile([128, L], bf16, name=f"kh{h}", tag=f"kh{h}")
              for h in range(HPC)]
        v_sb = qkpool.tile([128, 16, HPC, 97], bf16, name="v", tag="v")

        mqpool = ctx.enter_context(tc.tile_pool(name="mq", bufs=1))
        mask_q = {}
        for qc in range(NG):
            nmq = len(qc_mix[qc])
            if nmq:
                mask_q[qc] = (mqpool.tile([128, nmq, 512], bf16,
                                          name=f"mq{qc}", tag=f"mq{qc}"),
                              mix_idx[(qc_mix[qc][0], qc)])

        xpool = ctx.enter_context(tc.tile_pool(name="x", bufs=2))
        epool = ctx.enter_context(tc.tile_pool(name="ev", bufs=2))
        rpool = ctx.enter_context(tc.tile_pool(name="rp", bufs=1))
        estpool = ctx.enter_context(tc.tile_pool(name="est", bufs=3))
        npool = ctx.enter_context(tc.tile_pool(name="nrm", bufs=1))
        atpool = ctx.enter_context(tc.tile_pool(name="at", bufs=2))
        gpool = ctx.enter_context(tc.tile_pool(name="g", bufs=2))
        ypool = ctx.enter_context(tc.tile_pool(name="ye", bufs=2))

        psAB = ctx.enter_context(
            tc.tile_pool(name="psAB", bufs=2, space="PSUM"))
        ps_st = ctx.enter_context(
            tc.tile_pool(name="psST", bufs=2, space="PSUM"))
        ps_pv = ctx.enter_context(
            tc.tile_pool(name="psPV", bufs=2, space="PSUM"))
        psD = ctx.enter_context(tc.tile_pool(name="psD", bufs=2, space="PSUM"))

        dpool = ctx.enter_context(
            tc.tile_pool(name="dram", bufs=1, space="DRAM"))

        def emit_op(pend):
            """Return list of closures emitting the out-projection of a
            finished qc (consumes its g tile)."""
            g_sb, qc = pend
            calls = []
            for jj in range(4):
                lb = qc * 4 + jj
                box = {}

                def mk_alloc(box=box):
                    def f():
                        box["psy"] = psD.tile([128, DH], f32, name="psy",
                                              tag="d")
                    return f
                calls.append(mk_alloc())
                for k in range(KT):
                    def mk_mm(k=k, jj=jj, box=box):
                        def f():
                            nc.tensor.matmul(
                                box["psy"][:],
                                g_sb[:, k, jj * 128:(jj + 1) * 128],
                                wo_sb[:, k, :],
                                start=(k == 0), stop=False)
                        return f
                    calls.append(mk_mm())

                def mk_tail(lb=lb, box=box):
                    def f():
                        nc.tensor.matmul(box["psy"][:], ones_sb[:], cb_sb[:],
                                         start=False, stop=True)
                        ye = ypool.tile([128, DH], bf16, name="ye", tag="ye")
                        nc.vector.tensor_scalar_add(
                            ye[:], box["psy"][:], 0.0)
                        nc.gpsimd.dma_start(
                            out=d["y"][lb * 128:(lb + 1) * 128, :], in_=ye[:])
                    return f
                calls.append(mk_tail())
            return calls

        ag_pend = []   # [(ag_dram, qc)] awaiting SBUF reload
        op_pend = []   # [(g_sb, qc)] loaded, awaiting out-projection
        for rep in range(reps):
            # ---- per-rep reloads (overlap with previous rep's tail) -------
            # order: first k-half of wqk + group-0 x first, so the first
            # projection matmuls can start ~10us in
            nc.sync.dma_start(out=wqk_sb[:, 0:10, :], in_=wqk_r[:, 0:10, :])
            xa0 = xpool.tile([128, KT, NW], bf16, name="xa", tag="xa")
            nc.sync.dma_start(
                out=xa0[:, 0:10, :],
                in_=d["xTc"][0, 0:1280, :].rearrange("(t p) w -> p t w",
                                                     p=128))
            nc.sync.dma_start(out=wqk_sb[:, 10:20, :], in_=wqk_r[:, 10:20, :])
            nc.sync.dma_start(
                out=xa0[:, 10:20, :],
                in_=d["xTc"][0, 1280:2560, :].rearrange("(t p) w -> p t w",
                                                        p=128))
            nc.sync.dma_start(out=wv_sb[:, 0:10, :], in_=wv_r[:, 0:10, :])
            nc.sync.dma_start(out=wv_sb[:, 10:20, :], in_=wv_r[:, 10:20, :])
            nc.sync.dma_start(out=wo_sb[:], in_=wo_r[:])
            nc.sync.dma_start(out=cos_sb[:], in_=d["cos"][:])
            nc.sync.dma_start(out=sin_sb[:], in_=d["sin"][:])
            nc.sync.dma_start(out=bqk_sb[:], in_=d["bqk"][:])
            nc.sync.dma_start(out=cb_sb[:], in_=d["cb"][:])
            nc.sync.dma_start(out=ones_sb[:], in_=d["ones"][:])
            nc.sync.dma_start(out=tri_sb[:], in_=d["tri"][:])
            nc.gpsimd.dma_start(out=v_sb[:, :, :, HD:97], in_=d["vpad"][:])
            for qc in range(NG):
                if qc in mask_q:
                    mq, base = mask_q[qc]
                    nmq = mq.shape[1]
                    nc.sync.dma_start(out=mq[:],
                                      in_=mix_r[:, base:base + nmq, :])

            for g in range(NG):
                gsl = slice(g * NW, (g + 1) * NW)
                bounce = dpool.tile([DH, 512], bf16, name=f"bnc{rep}g{g}",
                                    tag=f"bnc{rep}g{g}")
                ag = dpool.tile([C, 512], bf16, name=f"agq{rep}g{g}",
                                tag=f"agq{rep}g{g}",
                                addr_space="Local" if sim_mode else "Shared")
                # ---- QKV projection for this group's 512 columns ----------
                if g == 0:
                    xa = xa0
                else:
                    xa = xpool.tile([128, KT, NW], bf16, name="xa", tag="xa")
                    nc.sync.dma_start(
                        out=xa[:, 0:10, :],
                        in_=d["xTc"][g, 0:1280, :]
                        .rearrange("(t p) w -> p t w", p=128))
                    nc.sync.dma_start(
                        out=xa[:, 10:20, :],
                        in_=d["xTc"][g, 1280:2560, :]
                        .rearrange("(t p) w -> p t w", p=128))

                ev = {}
                for m in range(5):
                    psm = psAB.tile([128, NW], f32, name="psm", tag="ab")
                    for k in range(KT):
                        nc.tensor.matmul(
                            psm[:],
                            wqk_sb[:, k, m * 128:(m + 1) * 128],
                            xa[:, k, :],
                            start=(k == 0), stop=(k == KT - 1))
                    e = epool.tile([128, NW], bf16, name=f"e{m}",
                                   tag=f"e{m}", bufs=2 if m < 2 else 1)
                    nc.vector.tensor_scalar_add(e[:], psm[:],
                                                bqk_sb[:, m:m + 1])
                    ev[m] = e
                    if m == 1:
                        # rope on ev0 (x1 rows) + ev1 (x2 rows), all bf16
                        co, si = cos_sb[:, gsl], sin_sb[:, gsl]
                        rt1 = rpool.tile([128, NW], bf16, name="rt1",
                                         tag="rt1")
                        rt2 = rpool.tile([128, NW], bf16, name="rt2",
                                         tag="rt2")
                        s1 = rpool.tile([128, NW], bf16, name="s1", tag="s1")
                        s2 = rpool.tile([128, NW], bf16, name="s2", tag="s2")
                        nc.vector.tensor_mul(rt1[:], ev[0][:], co)
                        nc.vector.tensor_mul(rt2[:], ev[1][:], si)
                        nc.vector.tensor_sub(s1[:], rt1[:], rt2[:])
                        nc.vector.tensor_mul(rt1[:], ev[0][:], si)
                        nc.vector.tensor_mul(rt2[:], ev[1][:], co)
                        nc.vector.tensor_add(s2[:], rt1[:], rt2[:])
                        for h in range(HPC):
                            nc.sync.dma_start(out=qh[h][0:16, gsl],
                                              in_=s1[16 * h:16 * h + 16, :])
                            nc.sync.dma_start(out=qh[h][16:32, gsl],
                                              in_=s2[16 * h:16 * h + 16, :])
                            nc.sync.dma_start(
                                out=kh[h][0:16, gsl],
                                in_=s1[64 + 16 * h:80 + 16 * h, :])
                            nc.sync.dma_start(
                                out=kh[h][16:32, gsl],
                                in_=s2[64 + 16 * h:80 + 16 * h, :])
                    if m >= 2:
                        for is_q, h, dr, sr, nr in _rest_runs(m):
                            dst = qh[h] if is_q else kh[h]
                            nc.sync.dma_start(out=dst[dr:dr + nr, gsl],
                                              in_=e[sr:sr + nr, :])
                for j in range(4):
                    psv = psAB.tile([128, NW], f32, name="psv", tag="ab")
                    for k in range(KT):
                        nc.tensor.matmul(
                            psv[:, 0:DH],
                            xa[:, k, j * 128:(j + 1) * 128],
                            wv_sb[:, k, :],
                            start=(k == 0), stop=(k == KT - 1))
                    nc.vector.tensor_scalar_add(
                        v_sb[:, 4 * g + j, :, 0:HD],
                        psv[:, 0:DH].rearrange("p (h dd) -> p h dd", h=HPC),
                        0.0)

                # reload the gathered attention output of group g-1 now:
                # the AllGather has had a full projection phase to complete,
                # so this SP-queue wait is short
                if ag_pend:
                    ag_prev, qc_prev = ag_pend.pop(0)
                    g_sb = gpool.tile([128, KT, 512], bf16, name="g", tag="g")
                    nc.sync.dma_start(
                        out=g_sb[:],
                        in_=ag_prev.rearrange("(t p) w -> p t w", p=128))
                    op_pend.append((g_sb, qc_prev))
                # ---- attention qc=g, out-proj of qc=g-2 interleaved -------
                opcalls = emit_op(op_pend.pop(0)) if len(op_pend) >= 2 else []
                blocks = [(h, kb) for h in range(HPC) for kb in live_kb[g]]
                nb = max(len(blocks), 1)
                nop = len(opcalls)
                emitted = 0
                pv = [None] * HPC
                for bi, (h, kb) in enumerate(blocks):
                    code, c0 = cls[kb][g]
                    if kb == live_kb[g][0]:
                        pv[h] = ps_pv.tile([97, 512], f32, name=f"pv{h}",
                                           tag="pv")
                    st = ps_st.tile([128, 512], f32, name="st", tag="st")
                    nc.tensor.matmul(
                        st[:, c0:],
                        kh[h][0:HD, kb * 128:(kb + 1) * 128],
                        qh[h][0:HD, g * NW + c0:(g + 1) * NW],
                        start=True, stop=True)
                    est = estpool.tile([128, 512], bf16, name="est",
                                       tag="est")
                    nc.scalar.activation(est[:, c0:], st[:, c0:], Exp)
                    if code == TRI:
                        tw = min(128, 512 - c0)
                        nc.vector.tensor_mul(est[:, c0:c0 + tw],
                                             est[:, c0:c0 + tw],
                                             tri_sb[:, 0:tw])
                    elif code == MIXG:
                        mq, base = mask_q[g]
                        idx = mix_idx[(kb, g)] - base
                        nc.vector.tensor_mul(est[:, c0:], est[:, c0:],
                                             mq[:, idx, c0:])
                    nc.tensor.matmul(
                        pv[h][:, c0:],
                        v_sb[:, kb, h, :],
                        est[:, c0:],
                        start=(kb == live_kb[g][0]),
                        stop=(kb == live_kb[g][-1]))
                    if kb == live_kb[g][-1]:
                        den = npool.tile([1, 512], f32, name="den", tag="den")
                        nc.vector.reciprocal(den[:], pv[h][96:97, :])
                        denb = npool.tile([HD, 512], f32, name="denb",
                                          tag="denb")
                        nc.gpsimd.partition_broadcast(denb[:], den[:])
                        attnq = atpool.tile([HD, 512], bf16, name="attnq",
                                            tag="attnq")
                        nc.vector.tensor_mul(attnq[:], pv[h][0:HD, :],
                                             denb[:])
                        nc.sync.dma_start(
                            out=bounce[h * HD:(h + 1) * HD, :],
                            in_=attnq[:])
                    # interleave pending out-proj quanta
                    want = (bi + 1) * nop // nb
                    while emitted < want:
                        opcalls[emitted]()
                        emitted += 1
                while emitted < nop:
                    opcalls[emitted]()
                    emitted += 1

                if sim_mode:
                    for rr in range(NCORES):
                        nc.sync.dma_start(
                            out=ag[rr * DH:(rr + 1) * DH, :],
                            in_=bounce[:])
                else:
                    nc.gpsimd.collective_compute(
                        "AllGather",
                        bypass,
                        replica_groups=[list(range(NCORES))],
                        ins=[bounce.opt()],
                        outs=[ag.opt()],
                    )
                ag_pend.append((ag, g))

        # drain: reload the final gather, then the remaining out-projections
        for ag_prev, qc_prev in ag_pend:
            g_sb = gpool.tile([128, KT, 512], bf16, name="g", tag="g")
            nc.sync.dma_start(
                out=g_sb[:],
                in_=ag_prev.rearrange("(t p) w -> p t w", p=128))
            op_pend.append((g_sb, qc_prev))
        for pend in op_pend:
            for call in emit_op(pend):
                call()


class Runner:
    """Builds + compiles once; callable repeatedly with per-core in_maps."""

    def __init__(self, mask_plan, reps=1):
        import jax
        from jax.sharding import Mesh, PartitionSpec
        from jax.experimental.shard_map import shard_map
        from concourse import mybir
        from concourse.bass2jax import (
            _bass_exec_p, install_neuronx_cc_hook, partition_id_tensor)

        self.jax = jax
        self.nc = _build(mask_plan, reps=reps)
        install_neuronx_cc_hook()
        nc = self.nc

        in_names, out_names, out_avals = [], [], []
        partition_name = (nc.partition_id_tensor.name
                          if nc.partition_id_tensor else None)
        for alloc in nc.m.functions[0].allocations:
            if not isinstance(alloc, mybir.MemoryLocationSet):
                continue
            name = alloc.memorylocations[0].name
            if alloc.kind == "ExternalInput":
                if name != partition_name:
                    in_names.append(name)
            elif alloc.kind == "ExternalOutput":
                out_names.append(name)
                shape = tuple(alloc.tensor_shape)
                dtype = mybir.dt.np(alloc.dtype)
                out_avals.append(jax.core.ShapedArray(shape, dtype))
        self.in_names = list(in_names)
        self.out_names = out_names
        self.out_avals = out_avals
        n_params = len(in_names)
        n_outs = len(out_avals)
        all_in_names = in_names + out_names
        if partition_name is not None:
            all_in_names.append(partition_name)

        def _body(*args):
            operands = list(args)
            if partition_name is not None:
                operands.append(partition_id_tensor())
            outs = _bass_exec_p.bind(
                *operands,
                out_avals=tuple(out_avals),
                in_names=tuple(all_in_names),
                out_names=tuple(out_names),
                lowering_input_output_aliases=(),
                sim_require_finite=True,
                sim_require_nnan=True,
                nc=nc,
            )
            return tuple(outs)

        devices = jax.devices()[:NCORES]
        mesh = Mesh(np.asarray(devices), ("core",))
        self.mesh = mesh
        in_specs = (PartitionSpec("core"),) * (n_params + n_outs)
        out_specs = (PartitionSpec("core"),) * n_outs
        self.fn = jax.jit(
            shard_map(_body, mesh=mesh, in_specs=in_specs,
                      out_specs=out_specs, check_rep=False),
            keep_unused=True)

    def prepare(self, in_maps):
        import jax
        from jax.sharding import NamedSharding, PartitionSpec
        sh = NamedSharding(self.mesh, PartitionSpec("core"))
        concat_in = [
            np.concatenate([np.asarray(m[name]) for m in in_maps], axis=0)
            for name in self.in_names
        ]
        self._dev_in = [jax.device_put(a, sh) for a in concat_in]
        jax.block_until_ready(self._dev_in)
        self._zero_sh = sh
        self._zcache = None

    def _zeros(self):
        import jax
        import jax.numpy as jnp
        if self._zcache is None:
            def mk(shape, dtype):
                return jax.jit(lambda: jnp.zeros(shape, dtype),
                               out_shardings=self._zero_sh)
            self._zcache = [
                mk((NCORES * a.shape[0], *a.shape[1:]), a.dtype)()
                for a in self.out_avals
            ]
            jax.block_until_ready(self._zcache)
        return self._zcache

    def run_prepared(self, fetch=True):
        import jax
        out = self.fn(*self._dev_in, *self._zeros())
        if not fetch:
            jax.block_until_ready(out)
            return None
        out = [np.asarray(o) for o in out]
        return [
            {name: out[i].reshape(NCORES, *self.out_avals[i].shape)[c]
             for i, name in enumerate(self.out_names)}
            for c in range(NCORES)
        ]

    def __call__(self, in_maps):
        self.prepare(in_maps)
        return self.run_prepared()

    def make_loop(self, n):
        """Jitted fn executing the NEFF n times sequentially (chained via a
        zeroed carry) inside one dispatch — for overhead-free timing."""
        import jax
        import jax.numpy as jnp
        from jax.sharding import PartitionSpec
        from jax.experimental.shard_map import shard_map
        from concourse.bass2jax import _bass_exec_p, partition_id_tensor

        nc = self.nc
        out_avals = self.out_avals
        in_names = self.in_names
        out_names = self.out_names
        partition_name = (nc.partition_id_tensor.name
                          if nc.partition_id_tensor else None)
        all_in_names = list(in_names) + list(out_names)
        if partition_name is not None:
            all_in_names.append(partition_name)

        def _loop(*args):
            ins = list(args)
            carry = [jnp.zeros(a.shape, a.dtype) for a in out_avals]
            outs = None
            for _ in range(n):
                operands = ins + carry
                if partition_name is not None:
                    operands.append(partition_id_tensor())
                outs = _bass_exec_p.bind(
                    *operands,
                    out_avals=tuple(out_avals),
                    in_names=tuple(all_in_names),
                    out_names=tuple(out_names),
                    lowering_input_output_aliases=(),
                    sim_require_finite=True,
                    sim_require_nnan=True,
                    nc=nc,
                )
                carry = [o * 0 for o in outs]
            return tuple(outs)

        n_params = len(in_names)
        in_specs = (PartitionSpec("core"),) * n_params
        out_specs = (PartitionSpec("core"),) * len(out_names)
        return jax.jit(shard_map(_loop, mesh=self.mesh, in_specs=in_specs,
                                 out_specs=out_specs, check_rep=False))

    def time_loop(self, n, iters=8):
        import time as _time
        import jax
        fn = self.make_loop(n)
        out = fn(*self._dev_in)
        jax.block_until_ready(out)
        ts = []
        for _ in range(iters):
            t0 = _time.perf_counter()
            out = fn(*self._dev_in)
            jax.block_until_ready(out)
            ts.append(_time.perf_counter() - t0)
        ts.sort()
        return ts[0], ts[len(ts) // 2]


def _mask_plan(maskT):
    """Classify each (kb 128-k-rows, qc 512-q-cols) block of the transposed
    mask. Returns ((cls, nmix), mixm) where cls[kb][qc] = (code, c0) and
    mixm holds exp(mask) blocks for generic-MIX blocks (qc-major order)."""
    tri = (np.arange(128)[:, None] <= np.arange(128)[None, :])
    cls = [[None] * NG for _ in range(16)]
    blocks = []
    for qc in range(NG):
        for kb in range(16):
            sub = maskT[kb * 128:(kb + 1) * 128, qc * 512:(qc + 1) * 512]
            with np.errstate(over="ignore"):
                expm = np.exp(sub.astype(np.float64)).astype(np.float32)
            if np.all(expm == 1.0):
                cls[kb][qc] = (ZERO, 0)
            elif np.all(expm == 0.0):
                cls[kb][qc] = (NEG, 0)
            else:
                nzcols = np.flatnonzero(expm.max(axis=0) > 0)
                c0 = int(nzcols[0])
                is_tri = (
                    c0 + 128 <= 512
                    and np.all(expm[:, :c0] == 0.0)
                    and np.array_equal(
                        expm[:, c0:c0 + 128], tri.astype(np.float32))
                    and np.all(expm[:, c0 + 128:] == 1.0))
                if is_tri:
                    cls[kb][qc] = (TRI, c0)
                else:
                    cls[kb][qc] = (MIXG, c0, len(blocks))
                    blocks.append(expm)
    # first live kb per qc must produce a full-width PV start
    for qc in range(NG):
        for kb in range(16):
            code = cls[kb][qc][0]
            if code == NEG:
                continue
            if code == ZERO:
                break
            if cls[kb][qc][1] != 0:
                if code == TRI:
                    sub = maskT[kb * 128:(kb + 1) * 128,
                                qc * 512:(qc + 1) * 512]
                    with np.errstate(over="ignore"):
                        expm = np.exp(sub.astype(np.float64)).astype(
                            np.float32)
                    cls[kb][qc] = (MIXG, 0, len(blocks))
                    blocks.append(expm)
                else:
                    cls[kb][qc] = (MIXG, 0, cls[kb][qc][2])
            break
    # renumber mix blocks qc-major in final class order
    import ml_dtypes
    final_blocks = []
    for qc in range(NG):
        for kb in range(16):
            if cls[kb][qc][0] == MIXG:
                final_blocks.append(blocks[cls[kb][qc][2]])
                cls[kb][qc] = (MIXG, cls[kb][qc][1])
    nmix = len(final_blocks)
    if final_blocks:
        mixm = np.stack(final_blocks, axis=0).astype(ml_dtypes.bfloat16)
    else:
        mixm = np.zeros((1, 128, 512), dtype=ml_dtypes.bfloat16)
    cls = tuple(tuple(r) for r in cls)
    return (cls, nmix), mixm


def _host_prep(x, Wqkv_w, Wqkv_b, out_w, out_b, mask):
    """Build per-core in_maps + mask plan (numpy only)."""
    import ml_dtypes
    x2 = np.ascontiguousarray(np.asarray(x, dtype=np.float32)[0])   # [L, C]
    xT = x2.T                                                        # [C, L]
    xTc = np.ascontiguousarray(
        np.stack([xT[:, g * NW:(g + 1) * NW] for g in range(NG)],
                 axis=0).astype(ml_dtypes.bfloat16))
    Wqkv_w = np.asarray(Wqkv_w, dtype=np.float32)
    Wqkv_b = np.asarray(Wqkv_b, dtype=np.float32)
    out_w = np.asarray(out_w, dtype=np.float32)
    out_b = np.asarray(out_b, dtype=np.float32)
    mask2 = np.asarray(mask, dtype=np.float32)[0, 0]                 # [L, L]
    maskT = np.ascontiguousarray(mask2.T)
    plan, mixm = _mask_plan(maskT)

    Wq, Wk, Wv = Wqkv_w[0:C], Wqkv_w[C:2 * C], Wqkv_w[2 * C:3 * C]
    bq, bk, bv = Wqkv_b[0:C], Wqkv_b[C:2 * C], Wqkv_b[2 * C:3 * C]

    pos = np.arange(L, dtype=np.float32)
    freq = np.exp(-np.arange(RD, dtype=np.float32) * (math.log(BASE) / RD))
    theta = pos[None, :] * freq[:, None]                             # [16, L]
    cos8 = np.ascontiguousarray(
        np.tile(np.cos(theta), (8, 1)).astype(ml_dtypes.bfloat16))
    sin8 = np.ascontiguousarray(
        np.tile(np.sin(theta), (8, 1)).astype(ml_dtypes.bfloat16))

    hidx = np.arange(HPC)[:, None]
    x1_idx = (80 * hidx + np.arange(RD)[None, :]).ravel()
    x2_idx = (80 * hidx + RD + np.arange(RD)[None, :]).ravel()
    rest_idx = (80 * hidx + ROT + np.arange(HD - ROT)[None, :]).ravel()

    ones128 = np.ones((1, 128), dtype=np.float32)
    vpad = np.zeros((128, 16, HPC, 17), dtype=np.float32)
    vpad[..., 16] = 1.0
    vpad = vpad.astype(ml_dtypes.bfloat16)
    tri128 = (np.arange(128)[:, None] <= np.arange(128)[None, :]).astype(
        ml_dtypes.bfloat16)

    in_maps = []
    for i in range(NCORES):
        rs = slice(DH * i, DH * (i + 1))
        Wq_i = Wq[rs] * SC
        bq_i = bq[rs] * SC
        Wk_i, bk_i, Wv_i = Wk[rs], bk[rs], Wv[rs]
        Wqk_i = np.concatenate([
            Wq_i[x1_idx], Wk_i[x1_idx],
            Wq_i[x2_idx], Wk_i[x2_idx],
            Wq_i[rest_idx], Wk_i[rest_idx]], axis=0)                 # [640, C]
        bqk_i = np.concatenate([
            bq_i[x1_idx], bk_i[x1_idx],
            bq_i[x2_idx], bk_i[x2_idx],
            bq_i[rest_idx], bk_i[rest_idx]], axis=0)
        wqk_t = np.ascontiguousarray(Wqk_i.T.astype(ml_dtypes.bfloat16))
        wv_t = np.ascontiguousarray(Wv_i.T.astype(ml_dtypes.bfloat16))
        Wo_i = out_w[rs]
        wo_t = np.ascontiguousarray(Wo_i.T.astype(ml_dtypes.bfloat16))
        cb_i = (out_b[rs] + Wo_i @ bv).astype(np.float32)[None, :]
        bqk_r = np.ascontiguousarray(bqk_i.reshape(5, 128).T)
        in_maps.append({
            "xTc": xTc,
            "wqk_t": wqk_t,
            "wv_t": wv_t,
            "wo_t": wo_t,
            "bqk": bqk_r,
            "cb": cb_i,
            "cos8": cos8,
            "sin8": sin8,
            "ones128": ones128,
            "vpad": vpad,
            "tri128": tri128,
            "mixm": mixm,
        })
    return in_maps, plan


def get_runner(mask_plan, reps=1):
    key = (mask_plan, reps)
    if key not in _RUNNERS:
        _RUNNERS[key] = Runner(mask_plan, reps=reps)
    return _RUNNERS[key]


def kernel(x, Wqkv_w, Wqkv_b, out_w, out_b, mask):
    in_maps, plan = _host_prep(x, Wqkv_w, Wqkv_b, out_w, out_b, mask)
    runner = get_runner(plan)
    results = runner(in_maps)
    y = np.concatenate(
        [np.asarray(results[i]["y"], dtype=np.float32)
         for i in range(NCORES)], axis=1)
    return y.reshape(1, L, C)
